# revision 6
# baseline (speedup 1.0000x reference)
"""Trainium2 Bass kernel for nn_EnsembleGCN (600-node episode GNN).

Strategy (8 NeuronCores, no ncfw collectives - they cost ~80us on this pool):
  Phase A (8 cores, SPMD): the small per-layer weights are replicated and the
  layer-1/2 GCN prefix is computed on every core (it is cheap dense matmul
  work); segment sums are done as dense matmuls against count matrices built
  from the edge lists (pure COO->dense format conversion on the host - all
  degree/normalization math happens on device). The expensive GATv2 pairwise
  attention (600x600x256 leaky_relu) is sharded by destination row: core c
  owns rows [75c, 75c+75) and computes e/softmax/messages/conv/fc and
  P_local = fc @ W_lab for its rows only.
  Host gathers the 8 P_local shards (pure concatenation).
  Phase B (1 core): final GCN layer out = D^-1/2 (A_fin+I) D^-1/2 P + b_lab.

Host-side work is restricted to marshalling: transposes, concatenation,
padding, dtype casts, and scattering edge lists into dense count matrices.
"""

import dataclasses
import numpy as np
import ml_dtypes

import concourse.bass as bass
import concourse.bacc as bacc
import concourse.tile as tile
import concourse.mybir as mybir
from concourse.bass_utils import run_bass_kernel_spmd
from concourse.masks import make_identity

dt = mybir.dt
AF = mybir.ActivationFunctionType
ALU = mybir.AluOpType

# ---------------------------------------------------------------- constants
N = 600          # nodes
C = 5            # classes
Q = 16           # queries per class
NQ = C * Q       # 80 query nodes
NS = N - NQ      # 520 support nodes
EMB = 128        # per-feature GCN embedding
FE = 256         # final embed size
CAT = 261        # 128 + 128 + 5
K = 3            # conv kernel
FC = 510         # 256 + 254
NCORES = 8
R = N // NCORES  # 75 rows per core
F_IN = 1024
SCH = 5          # node chunks of 128 (640 padded)
NPAD = 640

ALPHA = 0.2       # GATv2 attention leaky_relu slope
ALPHA_GCN = 0.01  # jax.nn.leaky_relu default used after GCN layers

# ---------------------------------------------------- custom DVE op: lrelu(x+b)
from concourse.dve_ops import DveOp
import concourse.dve_ops as dve_ops
from concourse.dve_spec import Spec, Src0, C0, C1, maxx


def _register_op(op):
    if any(o.name == op.name for o in dve_ops.OPS):
        return next(o for o in dve_ops.OPS if o.name == op.name)
    dve_ops.OPS.append(op)
    dve_ops.CUSTOM_DVE_SPECS[op.name] = op.spec
    row = dve_ops._CUSTOM_DVE_ROW_BASE + len(dve_ops.OPS) - 1
    assert row < 0x20
    dve_ops._SUB_OPCODE_FOR_NAME[op.name] = row
    return op


def _make_lrelu_bias():
    x = Src0 + C0
    spec = Spec(
        body=maxx(x, x * C1),
        reference=lambda in0, in1, s0, s1, imm2: np.maximum(
            in0 + s0, (in0 + s0) * s1
        ).astype(in0.dtype),
    )
    op = DveOp("LRELU_BIAS_ANT", spec, subdim=False, uops_sha={})
    _register_op(op)
    shas = {}
    for ver in ("v3", "v4"):
        try:
            op.compile(ver)
        except ValueError as e:
            shas[ver] = str(e).split(f"({ver}: ")[1].split(" ")[0]
    op2 = dataclasses.replace(op, uops_sha=shas, perf_en={"v3": True, "v4": True})
    dve_ops.OPS[[o.name for o in dve_ops.OPS].index(op.name)] = op2
    dve_ops.CUSTOM_DVE_SPECS[op.name] = op2.spec
    op2.compile("v3")
    return op2


LRELU_BIAS = _make_lrelu_bias()

# fraction of pairwise tiles produced on ScalarE (rest on VectorE custom op)
ACT_SHARE = 2  # every ACT_SHARE-th (i,kt) tile goes to ScalarE


# ================================================================ phase A ===
def build_phase_a(debug=False):
    nc = bacc.Bacc("TRN2", target_bir_lowering=False, debug=False,
                   num_devices=NCORES)

    def inp(name, shape, d=dt.float32):
        return nc.dram_tensor(name, shape, d, kind="ExternalInput").ap()

    f0T = inp("f0T", [F_IN, N])
    f1T = inp("f1T", [F_IN, N])
    w0 = inp("w0", [F_IN, EMB])
    w1 = inp("w1", [F_IN, EMB])
    ainT = inp("ainT", [NPAD, N])          # (A_in + I)^T counts, [s, d]
    ainTloc = inp("ainTloc", [NPAD, R])    # column shard for this core
    ctail = inp("ctail", [8, N])           # onehot_T (5) | ones (1) | 0 pad
    wib = inp("wib", [384, FE])            # W_in rows (no bias)
    wlb = inp("wlb", [384, FE])            # Wl rows + bl at row 261
    wrb = inp("wrb", [384, FE])            # Wr rows + br at row 261
    att = inp("att", [EMB, 2])             # att_a split into two columns
    mcr = inp("mcr", [R, N])               # (M_cr + I) rows for this core
    psel = inp("psel", [NPAD, R])          # one-hot row selector
    bf0 = inp("bf0", [EMB, 1])
    bf1 = inp("bf1", [EMB, 1])
    bin_ = inp("bin", [1, FE])
    bgat = inp("bgat", [1, FE])
    wcv = inp("wcv", [1, 4])               # conv w0,w1,w2, conv_b
    wlab = inp("wlab", [512, C])

    pout = nc.dram_tensor("pout", [R, C], dt.float32, kind="ExternalOutput").ap()
    dbg = {}
    if debug:
        for nm, shp in [("d_catedT", [128, 3 * N]), ("d_e", [R, N]),
                        ("d_exw", [R, N]), ("d_z", [R, FE]), ("d_g1", [R, FE]),
                        ("d_fc", [R, 512]), ("d_dinv", [128, SCH]),
                        ("d_hlT", [128, 2 * N]), ("d_hr", [128, R]),
                        ("d_t1", [128, SCH * FE])]:
            dbg[nm] = nc.dram_tensor(nm, shp, dt.float32,
                                     kind="ExternalOutput").ap()

    with tile.TileContext(nc) as tc:
        with tc.tile_pool(name="sb", bufs=1) as pool, \
             tc.tile_pool(name="sbw", bufs=3) as wpool, \
             tc.tile_pool(name="ps", bufs=1, space="PSUM") as psp, \
             tc.tile_pool(name="psA", bufs=2, space="PSUM") as pspA, \
             tc.tile_pool(name="dr", bufs=1, space="DRAM") as drp:

            # ---------------- resident loads
            ain_sb = pool.tile([128, SCH * N], dt.float32)     # 5 chunks
            for c in range(SCH):
                nc.sync.dma_start(ain_sb[:, c * N:(c + 1) * N],
                                  ainT[128 * c:128 * (c + 1), :])
            ainloc_sb = pool.tile([128, SCH * R], dt.float32)
            for c in range(SCH):
                nc.sync.dma_start(ainloc_sb[:, c * R:(c + 1) * R],
                                  ainTloc[128 * c:128 * (c + 1), :])
            psel_sb = pool.tile([128, SCH * R], dt.float32)
            for c in range(SCH):
                nc.sync.dma_start(psel_sb[:, c * R:(c + 1) * R],
                                  psel[128 * c:128 * (c + 1), :])
            wib_sb = pool.tile([128, 3 * FE], dt.float32)
            wlb_sb = pool.tile([128, 3 * FE], dt.float32)
            wrb_sb = pool.tile([128, 3 * FE], dt.float32)
            for w_sb, w_in in ((wib_sb, wib), (wlb_sb, wlb), (wrb_sb, wrb)):
                for c in range(3):
                    nc.sync.dma_start(w_sb[:, c * FE:(c + 1) * FE],
                                      w_in[128 * c:128 * (c + 1), :])
            att_sb = pool.tile([128, 2], dt.float32)
            nc.sync.dma_start(att_sb[:], att[:])
            mcr_sb = pool.tile([R, N], dt.float32)
            nc.sync.dma_start(mcr_sb[:], mcr[:])
            bf0_sb = pool.tile([EMB, 1], dt.float32)
            nc.sync.dma_start(bf0_sb[:], bf0[:])
            bf1_sb = pool.tile([EMB, 1], dt.float32)
            nc.sync.dma_start(bf1_sb[:], bf1[:])
            bin_sb = pool.tile([1, FE], dt.float32)
            nc.sync.dma_start(bin_sb[:], bin_[:])
            bgat_sb = pool.tile([1, FE], dt.float32)
            nc.sync.dma_start(bgat_sb[:], bgat[:])
            wcv_sb = pool.tile([1, 4], dt.float32)
            nc.sync.dma_start(wcv_sb[:], wcv[:])
            wlab_sb = pool.tile([128, 4 * C], dt.float32)
            for c in range(4):
                nc.sync.dma_start(wlab_sb[:, c * C:(c + 1) * C],
                                  wlab[128 * c:128 * (c + 1), :])

            ident = pool.tile([128, 128], dt.float32)
            make_identity(nc, ident[:])
            ones_col = pool.tile([128, 1], dt.float32)
            nc.vector.memset(ones_col[:], 1.0)

            # ---------------- deg / dinv for the in-graph
            deg_ps = pspA.tile([128, SCH], dt.float32, tag="mm", padded_shape=[128, 512])
            for m in range(SCH):
                nrow = min(128, N - 128 * m)
                for k in range(SCH):
                    nc.tensor.matmul(
                        deg_ps[0:nrow, m:m + 1],
                        ain_sb[:, k * N + 128 * m:k * N + 128 * m + nrow],
                        ones_col[:],
                        start=(k == 0), stop=(k == SCH - 1))
            deg_sb = pool.tile([128, SCH], dt.float32)
            nc.vector.tensor_scalar_max(deg_sb[:], deg_ps[:], 1.0)
            # dinv = rsqrt(deg), newton-refined
            rc = pool.tile([128, SCH], dt.float32)
            nc.vector.reciprocal(rc[:], deg_sb[:])
            sq0 = pool.tile([128, SCH], dt.float32)
            nc.scalar.activation(sq0[:], rc[:], AF.Sqrt)
            y2 = pool.tile([128, SCH], dt.float32)
            nc.vector.tensor_mul(y2[:], sq0[:], sq0[:])
            dy2 = pool.tile([128, SCH], dt.float32)
            nc.vector.tensor_mul(dy2[:], deg_sb[:], y2[:])
            cor = pool.tile([128, SCH], dt.float32)
            nc.vector.scalar_tensor_tensor(cor[:], dy2[:], -0.5, sq0[:],
                                           ALU.mult, ALU.mult)
            dinv = pool.tile([128, SCH], dt.float32)
            nc.vector.scalar_tensor_tensor(dinv[:], sq0[:], 1.5, cor[:],
                                           ALU.mult, ALU.add)
            if debug:
                nc.sync.dma_start(dbg["d_dinv"][:], dinv[:])

            # dinv as a broadcast row [128, 640] (via DRAM bounce reshape)
            dsc = drp.tile([128, SCH], dt.float32)
            nc.sync.dma_start(dsc[:], dinv[:])
            dinvrow = pool.tile([1, NPAD], dt.float32)
            nc.sync.dma_start(dinvrow[0:1, :], dsc[:].rearrange("p c -> c p"))
            dinvb = pool.tile([128, NPAD], dt.float32)
            nc.gpsimd.partition_broadcast(dinvb[:], dinvrow[0:1, :])

            # dinv for local rows [R, 1]
            dinvloc_ps = pspA.tile([R, 1], dt.float32, tag="mm", padded_shape=[128, 512])
            for k in range(SCH):
                nc.tensor.matmul(dinvloc_ps[:], psel_sb[:, k * R:(k + 1) * R],
                                 dinv[:, k:k + 1],
                                 start=(k == 0), stop=(k == SCH - 1))
            dinvloc = pool.tile([R, 1], dt.float32)
            nc.vector.tensor_copy(dinvloc[:], dinvloc_ps[:])

            # ---------------- layer 1: F0W = X @ W  (both features), scaled
            f0w = pool.tile([128, SCH * 2 * EMB], dt.float32)  # [s, 256] chunks
            nc.vector.memset(f0w[:], 0.0)
            for m in range(SCH):
                fps = pspA.tile([128, 2 * EMB], dt.float32, tag="mm", padded_shape=[128, 512])
                for fi, (fT, ww) in enumerate(((f0T, w0), (f1T, w1))):
                    for k in range(F_IN // 128):
                        fsb = wpool.tile([128, N], dt.float32, tag="fchunk")
                        nc.sync.dma_start(fsb[:], fT[128 * k:128 * (k + 1), :])
                        wsb = wpool.tile([128, EMB], dt.float32, tag="wchunk")
                        nc.sync.dma_start(wsb[:], ww[128 * k:128 * (k + 1), :])
                        ncols = min(128, N - 128 * m)
                        nc.tensor.matmul(
                            fps[0:ncols, fi * EMB:(fi + 1) * EMB],
                            fsb[:, 128 * m:128 * m + ncols],
                            wsb[:],
                            start=(k == 0), stop=(k == F_IN // 128 - 1))
                # scale rows by dinv_s while copying out of psum
                ncols = min(128, N - 128 * m)
                nc.vector.tensor_scalar_mul(
                    f0w[0:ncols, m * 2 * EMB:(m + 1) * 2 * EMB], fps[0:ncols, :],
                    dinv[0:ncols, m:m + 1])

            # ---------------- cated_T: feat-major H0/H1 + tail
            catedT = pool.tile([128, 3 * N], dt.float32)
            nc.vector.memset(catedT[:, 2 * N:3 * N], 0.0)
            nc.sync.dma_start(catedT[0:8, 2 * N:3 * N], ctail[:])
            for t in range(2):          # h0 then h1
                bcol = bf0_sb if t == 0 else bf1_sb
                for h in range(2):      # d halves of 300
                    hps = pspA.tile([128, 300], dt.float32, tag="mm", padded_shape=[128, 512])
                    for k in range(SCH):
                        nc.tensor.matmul(
                            hps[:],
                            f0w[:, k * 2 * EMB + t * EMB:
                                k * 2 * EMB + (t + 1) * EMB],
                            ain_sb[:, k * N + 300 * h:k * N + 300 * (h + 1)],
                            start=(k == 0), stop=(k == SCH - 1))
                        # lhsT = F0W chunk [s, f_t], rhs = ainT chunk [s, d]
                    hsc = wpool.tile([128, 300], dt.float32, tag="hsc")
                    nc.vector.tensor_tensor(hsc[:], hps[:],
                                            dinvb[:, 300 * h:300 * (h + 1)],
                                            ALU.mult)
                    nc.scalar.activation(
                        catedT[:, t * N + 300 * h:t * N + 300 * (h + 1)],
                        hsc[:], AF.Prelu, bias=bcol[:, 0:1], scale=1.0,
                        alpha=ALPHA_GCN)
            if debug:
                nc.sync.dma_start(dbg["d_catedT"][:], catedT[:])

            # ---------------- T1 = cated @ W_in (node-major), scaled by dinv_s
            t1s = pool.tile([128, SCH * FE], dt.float32)
            nc.vector.memset(t1s[:], 0.0)
            for m in range(SCH):
                ncols = min(128, N - 128 * m)
                tps = pspA.tile([128, FE], dt.float32, tag="mm", padded_shape=[128, 512])
                for k in range(3):
                    nc.tensor.matmul(
                        tps[0:ncols, :],
                        catedT[:, k * N + 128 * m:k * N + 128 * m + ncols],
                        wib_sb[:, k * FE:(k + 1) * FE],
                        start=(k == 0), stop=(k == 2))
                nc.vector.tensor_scalar_mul(t1s[0:ncols, m * FE:(m + 1) * FE],
                                            tps[0:ncols, :],
                                            dinv[0:ncols, m:m + 1])
            if debug:
                nc.sync.dma_start(dbg["d_t1"][:], t1s[:])

            # ---------------- hl node-major (messages)
            hl_nm = pool.tile([128, SCH * FE], dt.float32)
            nc.vector.memset(hl_nm[:], 0.0)
            for m in range(SCH):
                ncols = min(128, N - 128 * m)
                lps = pspA.tile([128, FE], dt.float32, tag="mm", padded_shape=[128, 512])
                for k in range(3):
                    nc.tensor.matmul(
                        lps[0:ncols, :],
                        catedT[:, k * N + 128 * m:k * N + 128 * m + ncols],
                        wlb_sb[:, k * FE:(k + 1) * FE],
                        start=(k == 0), stop=(k == 2))
                nc.vector.tensor_copy(hl_nm[0:ncols, m * FE:(m + 1) * FE],
                                      lps[0:ncols, :])

            # ---------------- hl feat-major (pairwise), cast to bf16
            hlT16 = pool.tile([128, 2 * N], dt.bfloat16)
            for t in range(2):
                for h in range(2):
                    lt_ps = pspA.tile([128, 300], dt.float32, tag="mm", padded_shape=[128, 512])
                    for k in range(3):
                        nc.tensor.matmul(
                            lt_ps[:],
                            wlb_sb[:, k * FE + t * 128:k * FE + (t + 1) * 128],
                            catedT[:, k * N + 300 * h:k * N + 300 * (h + 1)],
                            start=(k == 0), stop=(k == 2))
                    nc.vector.tensor_copy(
                        hlT16[:, t * N + 300 * h:t * N + 300 * (h + 1)],
                        lt_ps[:])
            if debug:
                hlTf = wpool.tile([128, 2 * N], dt.float32, tag="hlTf")
                nc.vector.tensor_copy(hlTf[:], hlT16[:])
                nc.sync.dma_start(dbg["d_hlT"][:], hlTf[:])

            # ---------------- hr for local rows, feat-major [256, 75]
            hr_nm = wpool.tile([128, SCH * FE], dt.float32, tag="hrnm")
            nc.vector.memset(hr_nm[:], 0.0)
            for m in range(SCH):
                ncols = min(128, N - 128 * m)
                rps = pspA.tile([128, FE], dt.float32, tag="mm", padded_shape=[128, 512])
                for k in range(3):
                    nc.tensor.matmul(
                        rps[0:ncols, :],
                        catedT[:, k * N + 128 * m:k * N + 128 * m + ncols],
                        wrb_sb[:, k * FE:(k + 1) * FE],
                        start=(k == 0), stop=(k == 2))
                nc.vector.tensor_copy(hr_nm[0:ncols, m * FE:(m + 1) * FE],
                                      rps[0:ncols, :])
            hrloc_ps = pspA.tile([R, FE], dt.float32, tag="mm", padded_shape=[128, 512])
            for k in range(SCH):
                nc.tensor.matmul(hrloc_ps[:], psel_sb[:, k * R:(k + 1) * R],
                                 hr_nm[:, k * FE:(k + 1) * FE],
                                 start=(k == 0), stop=(k == SCH - 1))
            hrloc_nm = wpool.tile([R, FE], dt.float32, tag="hrlocnm")
            nc.vector.tensor_copy(hrloc_nm[:], hrloc_ps[:])
            hrT = pool.tile([128, 2 * R], dt.float32)   # [feat, i] 2 chunks
            for t in range(2):
                tp_ps = pspA.tile([128, R], dt.float32, tag="mm", padded_shape=[128, 512])
                nc.tensor.transpose(tp_ps[:], hrloc_nm[:, t * 128:(t + 1) * 128],
                                    ident[0:R, 0:R])
                nc.vector.tensor_copy(hrT[:, t * R:(t + 1) * R], tp_ps[:])
            if debug:
                nc.sync.dma_start(dbg["d_hr"][0:128, 0:R], hrT[:, 0:R])

            # ---------------- pairwise e: R16 = lrelu(hlT + hr_i), PE-reduce
            zwin = pool.tile([128, 2 * 63], dt.bfloat16)
            nc.vector.memset(zwin[:], 0.0)
            nc.vector.tensor_copy(zwin[:, 31:32], att_sb[:, 0:1])
            nc.vector.tensor_copy(zwin[:, 63 + 31:63 + 32], att_sb[:, 1:2])
            e_ps = [psp.tile([128, 300], dt.float32, tag=f"eps{h}",
                             name=f"eps{h}")
                    for h in range(2)]
            groups = [(0, 0, 32), (1, 32, 64), (2, 64, 75)]
            tcount = 0
            with tc.tile_pool(name="rt", bufs=6) as rtp:
                for (g, i0, i1) in groups:
                    for i in range(i0, i1):
                        rloc = i - 32 * g
                        for kt in range(2):
                            r16 = rtp.tile([128, N], dt.bfloat16, tag="r16")
                            if tcount % ACT_SHARE == ACT_SHARE - 1:
                                nc.scalar.activation(
                                    r16[:], hlT16[:, kt * N:(kt + 1) * N],
                                    AF.Prelu, bias=hrT[:, kt * R + i:kt * R + i + 1],
                                    scale=1.0, alpha=ALPHA)
                            else:
                                nc.vector._custom_dve(
                                    LRELU_BIAS, out=r16[:],
                                    in0=hlT16[:, kt * N:(kt + 1) * N],
                                    s0=hrT[:, kt * R + i:kt * R + i + 1],
                                    s1=ALPHA)
                            tcount += 1
                            for h in range(2):
                                nc.tensor.matmul(
                                    e_ps[h][32 * g:32 * g + 32, :],
                                    zwin[:, 63 * kt + 31 - rloc:
                                         63 * kt + 63 - rloc],
                                    r16[:, 300 * h:300 * (h + 1)],
                                    start=(rloc == 0 and kt == 0),
                                    stop=(rloc == (i1 - i0 - 1) and kt == 1),
                                    tile_position=(0, 32 * g))

            # ---------------- softmax over masked rows
            e_sb = pool.tile([R, N], dt.float32)
            for h in range(2):
                nc.vector.tensor_copy(e_sb[:, 300 * h:300 * (h + 1)],
                                      e_ps[h][0:R, :])
            if debug:
                nc.sync.dma_start(dbg["d_e"][:], e_sb[:])
            teq = wpool.tile([R, N], dt.float32, tag="teq")
            nc.vector.tensor_scalar(teq[:], mcr_sb[:], 0.0, None, ALU.is_equal)
            e_m = wpool.tile([R, N], dt.float32, tag="em")
            nc.vector.scalar_tensor_tensor(e_m[:], teq[:], -1e30, e_sb[:],
                                           ALU.mult, ALU.add)
            me = pool.tile([R, 1], dt.float32)
            nc.vector.tensor_reduce(me[:], e_m[:], mybir.AxisListType.X, ALU.max)
            nme = pool.tile([R, 1], dt.float32)
            nc.vector.tensor_scalar_mul(nme[:], me[:], -1.0)
            ex = wpool.tile([R, N], dt.float32, tag="ex")
            nc.scalar.activation(ex[:], e_m[:], AF.Exp, bias=nme[:, 0:1])
            exw = pool.tile([R, N], dt.float32)
            nc.vector.tensor_tensor(exw[:], ex[:], mcr_sb[:], ALU.mult)
            if debug:
                nc.sync.dma_start(dbg["d_exw"][:], exw[:])
            den = pool.tile([R, 1], dt.float32)
            nc.vector.tensor_reduce(den[:], exw[:], mybir.AxisListType.X,
                                    ALU.add)
            rec = pool.tile([R, 1], dt.float32)
            nc.vector.reciprocal(rec[:], den[:])

            # ---------------- messages: z = (exw @ hl) * rec + bgat
            exwT = pool.tile([128, SCH * R], dt.float32)
            nc.vector.memset(exwT[:], 0.0)
            for cidx in range(SCH):
                wdt = min(128, N - 128 * cidx)
                xp_ps = pspA.tile([128, R], dt.float32, tag="mm", padded_shape=[128, 512])
                nc.tensor.transpose(xp_ps[0:wdt, :],
                                    exw[:, 128 * cidx:128 * cidx + wdt],
                                    ident[0:R, 0:R])
                nc.vector.tensor_copy(exwT[0:wdt, cidx * R:(cidx + 1) * R],
                                      xp_ps[0:wdt, :])
            bgat_b = pool.tile([128, FE], dt.float32)
            nc.gpsimd.partition_broadcast(bgat_b[:], bgat_sb[0:1, :])
            z_ps = pspA.tile([R, FE], dt.float32, tag="mm", padded_shape=[128, 512])
            for cidx in range(SCH):
                nc.tensor.matmul(z_ps[:], exwT[:, cidx * R:(cidx + 1) * R],
                                 hl_nm[:, cidx * FE:(cidx + 1) * FE],
                                 start=(cidx == 0), stop=(cidx == SCH - 1))
            z_sb = pool.tile([R, FE], dt.float32)
            nc.vector.scalar_tensor_tensor(z_sb[:], z_ps[:], rec[:, 0:1],
                                           bgat_b[0:R, :], ALU.mult, ALU.add)
            if debug:
                nc.sync.dma_start(dbg["d_z"][:], z_sb[:])

            # ---------------- elu + 1
            zn = wpool.tile([R, FE], dt.float32, tag="zn")
            nc.vector.tensor_scalar_min(zn[:], z_sb[:], 0.0)
            ez = wpool.tile([R, FE], dt.float32, tag="ez")
            nc.scalar.activation(ez[:], zn[:], AF.Exp)
            g1 = pool.tile([R, FE], dt.float32)
            nc.vector.scalar_tensor_tensor(g1[:], z_sb[:], 0.0, ez[:],
                                           ALU.max, ALU.add)
            if debug:
                nc.sync.dma_start(dbg["d_g1"][:], g1[:])

            # ---------------- conv1d (K=3) + sigmoid, write into fc tile
            fc = pool.tile([R, 512], dt.float32)
            nc.vector.memset(fc[:, 510:512], 0.0)
            wb = pool.tile([128, 4], dt.float32)
            nc.gpsimd.partition_broadcast(wb[:], wcv_sb[0:1, :])
            s3 = pool.tile([128, 1], dt.float32)
            nc.vector.tensor_reduce(s3[:], wb[:, 0:3], mybir.AxisListType.X,
                                    ALU.add)
            cbp = pool.tile([128, 1], dt.float32)
            nc.vector.tensor_tensor(cbp[:], wb[:, 3:4], s3[:], ALU.subtract)
            a1 = wpool.tile([R, FE - K + 1], dt.float32, tag="a1")
            nc.vector.tensor_scalar_mul(a1[:], g1[:, 0:FE - K + 1], wb[0:R, 0:1])
            a2 = wpool.tile([R, FE - K + 1], dt.float32, tag="a2")
            nc.vector.scalar_tensor_tensor(a2[:], g1[:, 1:FE - K + 2],
                                           wb[0:R, 1:2], a1[:], ALU.mult,
                                           ALU.add)
            a3 = wpool.tile([R, FE - K + 1], dt.float32, tag="a3")
            nc.vector.scalar_tensor_tensor(a3[:], g1[:, 2:FE - K + 3],
                                           wb[0:R, 2:3], a2[:], ALU.mult,
                                           ALU.add)
            nc.scalar.activation(fc[:, FE:FE + FE - K + 1], a3[:], AF.Sigmoid,
                                 bias=cbp[0:R, 0:1])

            # ---------------- gcn_emb for local rows -> fc[:, 0:256]
            bin_b = pool.tile([128, FE], dt.float32)
            nc.gpsimd.partition_broadcast(bin_b[:], bin_sb[0:1, :])
            g_ps = pspA.tile([R, FE], dt.float32, tag="mm", padded_shape=[128, 512])
            for k in range(SCH):
                nc.tensor.matmul(g_ps[:], ainloc_sb[:, k * R:(k + 1) * R],
                                 t1s[:, k * FE:(k + 1) * FE],
                                 start=(k == 0), stop=(k == SCH - 1))
            gtmp = wpool.tile([R, FE], dt.float32, tag="gtmp")
            nc.vector.scalar_tensor_tensor(gtmp[:], g_ps[:], dinvloc[:, 0:1],
                                           bin_b[0:R, :], ALU.mult, ALU.add)
            nc.scalar.activation(fc[:, 0:FE], gtmp[:], AF.Prelu, bias=0.0,
                                 scale=1.0, alpha=ALPHA_GCN)
            if debug:
                nc.sync.dma_start(dbg["d_fc"][:], fc[:])

            # ---------------- P = fc @ W_lab
            fcT = pool.tile([128, 4 * R], dt.float32)
            for cidx in range(4):
                fp_ps = pspA.tile([128, R], dt.float32, tag="mm", padded_shape=[128, 512])
                nc.tensor.transpose(fp_ps[:],
                                    fc[:, 128 * cidx:128 * (cidx + 1)],
                                    ident[0:R, 0:R])
                nc.vector.tensor_copy(fcT[:, cidx * R:(cidx + 1) * R],
                                      fp_ps[:])
            p_ps = pspA.tile([R, C], dt.float32, tag="mm", padded_shape=[128, 512])
            for cidx in range(4):
                nc.tensor.matmul(p_ps[:], fcT[:, cidx * R:(cidx + 1) * R],
                                 wlab_sb[:, cidx * C:(cidx + 1) * C],
                                 start=(cidx == 0), stop=(cidx == 3))
            p_sb = pool.tile([R, C], dt.float32)
            nc.vector.tensor_copy(p_sb[:], p_ps[:])
            nc.sync.dma_start(pout[:], p_sb[:])

    nc.compile()
    return nc


# ================================================================ phase B ===
def build_phase_b():
    nc = bacc.Bacc("TRN2", target_bir_lowering=False, debug=False,
                   num_devices=1)
    pfull = nc.dram_tensor("pfull", [NPAD, C], dt.float32,
                           kind="ExternalInput").ap()
    afinT = nc.dram_tensor("afinT", [NPAD, N], dt.float32,
                           kind="ExternalInput").ap()
    blab = nc.dram_tensor("blab", [1, C], dt.float32,
                          kind="ExternalInput").ap()
    out = nc.dram_tensor("out", [N, C], dt.float32,
                         kind="ExternalOutput").ap()

    with tile.TileContext(nc) as tc:
        with tc.tile_pool(name="sb", bufs=1) as pool, \
             tc.tile_pool(name="ps", bufs=2, space="PSUM") as psp:
            af_sb = pool.tile([128, SCH * N], dt.float32)
            for c in range(SCH):
                nc.sync.dma_start(af_sb[:, c * N:(c + 1) * N],
                                  afinT[128 * c:128 * (c + 1), :])
            p_sb = pool.tile([128, SCH * C], dt.float32)
            for c in range(SCH):
                nc.sync.dma_start(p_sb[:, c * C:(c + 1) * C],
                                  pfull[128 * c:128 * (c + 1), :])
            blab_sb = pool.tile([1, C], dt.float32)
            nc.sync.dma_start(blab_sb[:], blab[:])
            ones_col = pool.tile([128, 1], dt.float32)
            nc.vector.memset(ones_col[:], 1.0)

            deg_ps = psp.tile([128, SCH], dt.float32, tag="mm", padded_shape=[128, 512])
            for m in range(SCH):
                nrow = min(128, N - 128 * m)
                for k in range(SCH):
                    nc.tensor.matmul(
                        deg_ps[0:nrow, m:m + 1],
                        af_sb[:, k * N + 128 * m:k * N + 128 * m + nrow],
                        ones_col[:],
                        start=(k == 0), stop=(k == SCH - 1))
            deg_sb = pool.tile([128, SCH], dt.float32)
            nc.vector.tensor_scalar_max(deg_sb[:], deg_ps[:], 1.0)
            rc = pool.tile([128, SCH], dt.float32)
            nc.vector.reciprocal(rc[:], deg_sb[:])
            sq0 = pool.tile([128, SCH], dt.float32)
            nc.scalar.activation(sq0[:], rc[:], AF.Sqrt)
            y2 = pool.tile([128, SCH], dt.float32)
            nc.vector.tensor_mul(y2[:], sq0[:], sq0[:])
            dy2 = pool.tile([128, SCH], dt.float32)
            nc.vector.tensor_mul(dy2[:], deg_sb[:], y2[:])
            cor = pool.tile([128, SCH], dt.float32)
            nc.vector.scalar_tensor_tensor(cor[:], dy2[:], -0.5, sq0[:],
                                           ALU.mult, ALU.mult)
            dinv = pool.tile([128, SCH], dt.float32)
            nc.vector.scalar_tensor_tensor(dinv[:], sq0[:], 1.5, cor[:],
                                           ALU.mult, ALU.add)

            ps_sc = pool.tile([128, SCH * C], dt.float32)
            for c in range(SCH):
                nc.vector.tensor_scalar(ps_sc[:, c * C:(c + 1) * C],
                                        p_sb[:, c * C:(c + 1) * C],
                                        dinv[:, c:c + 1], None, ALU.mult)
            blab_b = pool.tile([128, C], dt.float32)
            nc.gpsimd.partition_broadcast(blab_b[:], blab_sb[0:1, :])
            for m in range(SCH):
                nrow = min(128, N - 128 * m)
                o_ps = psp.tile([128, C], dt.float32, tag="mm", padded_shape=[128, 512])
                for k in range(SCH):
                    nc.tensor.matmul(
                        o_ps[0:nrow, :],
                        af_sb[:, k * N + 128 * m:k * N + 128 * m + nrow],
                        ps_sc[:, k * C:(k + 1) * C],
                        start=(k == 0), stop=(k == SCH - 1))
                o_sb = pool.tile([128, C], dt.float32, tag="osb", bufs=2)
                nc.vector.scalar_tensor_tensor(o_sb[0:nrow, :], o_ps[0:nrow, :],
                                               dinv[0:nrow, m:m + 1],
                                               blab_b[0:nrow, :],
                                               ALU.mult, ALU.add)
                nc.sync.dma_start(out[128 * m:128 * m + nrow, :],
                                  o_sb[0:nrow, :])
    nc.compile()
    return nc


# ============================================================ host prep ====
def _dense_count(src, dst, add_eye=True):
    """COO edge list -> dense count matrix M[src, dst] (format conversion)."""
    m = np.zeros((NPAD, N), np.float32)
    np.add.at(m, (np.asarray(src, np.int64), np.asarray(dst, np.int64)), 1.0)
    if add_eye:
        m[np.arange(N), np.arange(N)] += 1.0
    return m


def _pad_rows(a, rows):
    out = np.zeros((rows, a.shape[1]), np.float32)
    out[:a.shape[0]] = a
    return out


_CACHE = {}


def _get_programs(debug=False):
    key = ("progs", debug)
    if key not in _CACHE:
        _CACHE[key] = (build_phase_a(debug=debug), build_phase_b())
    return _CACHE[key]


def prep_a_inputs(inputs):
    f32 = np.float32
    feats0 = np.asarray(inputs["features0"], f32)
    feats1 = np.asarray(inputs["features1"], f32)
    labels = np.asarray(inputs["labels"])
    ainT = _dense_count(inputs["in_src"], inputs["in_dst"])
    mcr_full = np.zeros((N, N), f32)
    np.add.at(mcr_full, (np.asarray(inputs["cr_dst"], np.int64),
                         np.asarray(inputs["cr_src"], np.int64)), 1.0)
    mcr_full[np.arange(N), np.arange(N)] += 1.0

    onehot = np.zeros((C, N), f32)
    sup = np.asarray(labels[:NS], np.int64)
    onehot[sup, np.arange(NS)] = 1.0
    onehot[:, NS:] = 1.0 / C
    ctail = np.zeros((8, N), f32)
    ctail[0:C] = onehot
    ctail[C] = 1.0

    def wstack(w, b=None):
        out = np.zeros((384, FE), f32)
        out[0:CAT] = np.asarray(w, f32)
        if b is not None:
            out[CAT] = np.asarray(b, f32)
        return out

    att2 = np.stack([np.asarray(inputs["att_a"][0:128], f32),
                     np.asarray(inputs["att_a"][128:256], f32)], axis=1)
    wcv = np.concatenate([np.asarray(inputs["conv_w"], f32).reshape(-1),
                          np.asarray(inputs["conv_b"], f32).reshape(-1)])
    wlab = np.zeros((512, C), f32)
    wlab[0:FC] = np.asarray(inputs["W_lab"], f32)

    base = {
        "f0T": np.ascontiguousarray(feats0.T),
        "f1T": np.ascontiguousarray(feats1.T),
        "w0": np.asarray(inputs["W_f0"], f32),
        "w1": np.asarray(inputs["W_f1"], f32),
        "ainT": ainT,
        "ctail": ctail,
        "wib": wstack(inputs["W_in"]),
        "wlb": wstack(inputs["Wl"], inputs["bl"]),
        "wrb": wstack(inputs["Wr"], inputs["br"]),
        "att": att2,
        "bf0": np.asarray(inputs["b_f0"], f32).reshape(EMB, 1),
        "bf1": np.asarray(inputs["b_f1"], f32).reshape(EMB, 1),
        "bin": np.asarray(inputs["b_in"], f32).reshape(1, FE),
        "bgat": np.asarray(inputs["b_gat"], f32).reshape(1, FE),
        "wcv": wcv.reshape(1, 4),
        "wlab": wlab,
    }
    in_maps = []
    eye_sel = np.zeros((NPAD, R), f32)
    for cix in range(NCORES):
        m = dict(base)
        rows = slice(R * cix, R * (cix + 1))
        m["ainTloc"] = np.ascontiguousarray(ainT[:, rows])
        m["mcr"] = np.ascontiguousarray(mcr_full[rows, :])
        sel = np.zeros((NPAD, R), f32)
        sel[np.arange(R * cix, R * (cix + 1)), np.arange(R)] = 1.0
        m["psel"] = sel
        in_maps.append(m)
    return in_maps


def kernel(**inputs) -> np.ndarray:
    nca, ncb = _get_programs()
    in_maps = prep_a_inputs(inputs)
    res_a = run_bass_kernel_spmd(nca, in_maps, core_ids=list(range(NCORES)))
    p_full = np.concatenate([res_a.results[cix]["pout"]
                             for cix in range(NCORES)], axis=0)
    afinT = _dense_count(inputs["fin_src"], inputs["fin_dst"])
    in_b = {
        "pfull": _pad_rows(p_full, NPAD),
        "afinT": afinT,
        "blab": np.asarray(inputs["b_lab"], np.float32).reshape(1, C),
    }
    res_b = run_bass_kernel_spmd(ncb, [in_b], core_ids=[0])
    return np.asarray(res_b.results[0]["out"], np.float32)


# revision 10
# speedup vs baseline: 1.4724x; 1.4724x over previous
"""Trainium2 Bass kernel for nn_EnsembleGCN (600-node episode GNN).

Strategy (8 NeuronCores, no ncfw collectives - they cost ~80us on this pool):
  Phase A (8 cores, SPMD): the small per-layer weights are replicated and the
  layer-1/2 GCN prefix is computed on every core (it is cheap dense matmul
  work); segment sums are done as dense matmuls against count matrices built
  from the edge lists (pure COO->dense format conversion on the host - all
  degree/normalization math happens on device). The expensive GATv2 pairwise
  attention (600x600x256 leaky_relu) is sharded by destination row: core c
  owns rows [75c, 75c+75) and computes e/softmax/messages/conv/fc and
  P_local = fc @ W_lab for its rows only.
  Host gathers the 8 P_local shards (pure concatenation).
  Phase B (1 core): final GCN layer out = D^-1/2 (A_fin+I) D^-1/2 P + b_lab.

Host-side work is restricted to marshalling: transposes, concatenation,
padding, dtype casts, and scattering edge lists into dense count matrices.
"""

import dataclasses
import numpy as np
import ml_dtypes

import concourse.bass as bass
import concourse.bacc as bacc
import concourse.tile as tile
import concourse.mybir as mybir
from concourse.bass_utils import run_bass_kernel_spmd
from concourse.masks import make_identity

dt = mybir.dt
AF = mybir.ActivationFunctionType
ALU = mybir.AluOpType

# ---------------------------------------------------------------- constants
N = 600          # nodes
C = 5            # classes
Q = 16           # queries per class
NQ = C * Q       # 80 query nodes
NS = N - NQ      # 520 support nodes
EMB = 128        # per-feature GCN embedding
FE = 256         # final embed size
CAT = 261        # 128 + 128 + 5
K = 3            # conv kernel
FC = 510         # 256 + 254
NCORES = 8
R = N // NCORES  # 75 rows per core
F_IN = 1024
SCH = 5          # node chunks of 128 (640 padded)
NPAD = 640

ALPHA = 0.2       # GATv2 attention leaky_relu slope
ALPHA_GCN = 0.01  # jax.nn.leaky_relu default used after GCN layers

# ---------------------------------------------------- custom DVE op: lrelu(x+b)
from concourse.dve_ops import DveOp
import concourse.dve_ops as dve_ops
from concourse.dve_spec import Spec, Src0, C0, C1, maxx


def _register_op(op):
    if any(o.name == op.name for o in dve_ops.OPS):
        return next(o for o in dve_ops.OPS if o.name == op.name)
    dve_ops.OPS.append(op)
    dve_ops.CUSTOM_DVE_SPECS[op.name] = op.spec
    row = dve_ops._CUSTOM_DVE_ROW_BASE + len(dve_ops.OPS) - 1
    assert row < 0x20
    dve_ops._SUB_OPCODE_FOR_NAME[op.name] = row
    return op


def _make_lrelu_bias():
    x = Src0 + C0
    spec = Spec(
        body=maxx(x, x * C1),
        reference=lambda in0, in1, s0, s1, imm2: np.maximum(
            in0 + s0, (in0 + s0) * s1
        ).astype(in0.dtype),
    )
    op = DveOp("LRELU_BIAS_ANT", spec, subdim=False, uops_sha={})
    _register_op(op)
    shas = {}
    for ver in ("v3", "v4"):
        try:
            op.compile(ver)
        except ValueError as e:
            shas[ver] = str(e).split(f"({ver}: ")[1].split(" ")[0]
    op2 = dataclasses.replace(op, uops_sha=shas, perf_en={"v3": True, "v4": True})
    dve_ops.OPS[[o.name for o in dve_ops.OPS].index(op.name)] = op2
    dve_ops.CUSTOM_DVE_SPECS[op.name] = op2.spec
    op2.compile("v3")
    return op2


LRELU_BIAS = _make_lrelu_bias()

# fraction of pairwise tiles produced on ScalarE (rest on VectorE custom op)
ACT_SHARE = 2  # every ACT_SHARE-th (i,kt) tile goes to ScalarE


# ================================================================ phase A ===
def build_phase_a(debug=False):
    nc = bacc.Bacc("TRN2", target_bir_lowering=False, debug=False,
                   num_devices=NCORES)

    def inp(name, shape, d=dt.float32):
        return nc.dram_tensor(name, shape, d, kind="ExternalInput").ap()

    f0T = inp("f0T", [F_IN, N])
    f1T = inp("f1T", [F_IN, N])
    w0 = inp("w0", [F_IN, EMB])
    w1 = inp("w1", [F_IN, EMB])
    ainT = inp("ainT", [NPAD, N])          # (A_in + I)^T counts, [s, d]
    ainTloc = inp("ainTloc", [NPAD, R])    # column shard for this core
    ctail = inp("ctail", [8, N])           # onehot_T (5) | ones (1) | 0 pad
    wall = inp("wall", [384, 3 * FE])      # [W_in | Wl+bl | Wr+br]
    att = inp("att", [EMB, 2])             # att_a split into two columns
    mcr = inp("mcr", [R, N])               # (M_cr + I) rows for this core
    psel = inp("psel", [NPAD, R])          # one-hot row selector
    bf0 = inp("bf0", [EMB, 1])
    bf1 = inp("bf1", [EMB, 1])
    bin_ = inp("bin", [1, FE])
    bgat = inp("bgat", [1, FE])
    wcv = inp("wcv", [1, 4])               # conv w0,w1,w2, conv_b
    wlab = inp("wlab", [512, C])

    pout = nc.dram_tensor("pout", [R, C], dt.float32, kind="ExternalOutput").ap()
    dbg = {}
    if debug:
        for nm, shp in [("d_catedT", [128, 3 * N]), ("d_e", [R, N]),
                        ("d_exw", [R, N]), ("d_z", [R, FE]), ("d_g1", [R, FE]),
                        ("d_fc", [R, 512]), ("d_dinv", [128, SCH]),
                        ("d_hlT", [128, 2 * N]), ("d_hr", [128, R]),
                        ("d_t1", [128, SCH * FE])]:
            dbg[nm] = nc.dram_tensor(nm, shp, dt.float32,
                                     kind="ExternalOutput").ap()

    with tile.TileContext(nc) as tc:
        with tc.tile_pool(name="sb", bufs=1) as pool, \
             tc.tile_pool(name="sbw", bufs=3) as wpool, \
             tc.tile_pool(name="ps", bufs=1, space="PSUM") as psp, \
             tc.tile_pool(name="psA", bufs=3, space="PSUM") as pspA, \
             tc.tile_pool(name="dr", bufs=1, space="DRAM") as drp:

            # ---------------- resident loads
            ain_sb = pool.tile([128, SCH * N], dt.float32)     # 5 chunks
            for c in range(SCH):
                nc.sync.dma_start(ain_sb[:, c * N:(c + 1) * N],
                                  ainT[128 * c:128 * (c + 1), :])
            ainloc_sb = pool.tile([128, SCH * R], dt.float32)
            for c in range(SCH):
                nc.sync.dma_start(ainloc_sb[:, c * R:(c + 1) * R],
                                  ainTloc[128 * c:128 * (c + 1), :])
            psel_sb = pool.tile([128, SCH * R], dt.float32)
            for c in range(SCH):
                nc.sync.dma_start(psel_sb[:, c * R:(c + 1) * R],
                                  psel[128 * c:128 * (c + 1), :])
            wall_sb = pool.tile([128, 3 * 3 * FE], dt.float32)
            for c in range(3):
                nc.sync.dma_start(wall_sb[:, c * 768:(c + 1) * 768],
                                  wall[128 * c:128 * (c + 1), :])
            # feats + layer-1 weights resident (load once)
            f0sb = pool.tile([128, 8 * N], dt.float32)
            f1sb = pool.tile([128, 8 * N], dt.float32)
            w0sb = pool.tile([128, 8 * EMB], dt.float32)
            w1sb = pool.tile([128, 8 * EMB], dt.float32)
            for k in range(8):
                nc.sync.dma_start(f0sb[:, k * N:(k + 1) * N],
                                  f0T[128 * k:128 * (k + 1), :])
                nc.sync.dma_start(f1sb[:, k * N:(k + 1) * N],
                                  f1T[128 * k:128 * (k + 1), :])
                nc.sync.dma_start(w0sb[:, k * EMB:(k + 1) * EMB],
                                  w0[128 * k:128 * (k + 1), :])
                nc.sync.dma_start(w1sb[:, k * EMB:(k + 1) * EMB],
                                  w1[128 * k:128 * (k + 1), :])
            att_sb = pool.tile([128, 2], dt.float32)
            nc.sync.dma_start(att_sb[:], att[:])
            mcr_sb = pool.tile([R, N], dt.float32)
            nc.sync.dma_start(mcr_sb[:], mcr[:])
            bf0_sb = pool.tile([EMB, 1], dt.float32)
            nc.sync.dma_start(bf0_sb[:], bf0[:])
            bf1_sb = pool.tile([EMB, 1], dt.float32)
            nc.sync.dma_start(bf1_sb[:], bf1[:])
            bin_sb = pool.tile([1, FE], dt.float32)
            nc.sync.dma_start(bin_sb[:], bin_[:])
            bgat_sb = pool.tile([1, FE], dt.float32)
            nc.sync.dma_start(bgat_sb[:], bgat[:])
            wcv_sb = pool.tile([1, 4], dt.float32)
            nc.sync.dma_start(wcv_sb[:], wcv[:])
            wlab_sb = pool.tile([128, 4 * C], dt.float32)
            for c in range(4):
                nc.sync.dma_start(wlab_sb[:, c * C:(c + 1) * C],
                                  wlab[128 * c:128 * (c + 1), :])

            ident = pool.tile([128, 128], dt.float32)
            make_identity(nc, ident[:])
            ones_col = pool.tile([128, 1], dt.float32)
            nc.vector.memset(ones_col[:], 1.0)

            # ---------------- deg / dinv for the in-graph (ones-row trick)
            deg_psh = [pspA.tile([1, 300], dt.float32, tag="mm",
                                 padded_shape=[128, 512], name=f"degps{h}")
                       for h in range(2)]
            for h in range(2):
                for k in range(SCH):
                    nc.tensor.matmul(
                        deg_psh[h][0:1, :],
                        ones_col[:],
                        ain_sb[:, k * N + 300 * h:k * N + 300 * (h + 1)],
                        start=(k == 0), stop=(k == SCH - 1))
            dgd_sb = pool.tile([1, NPAD], dt.float32)
            nc.vector.memset(dgd_sb[:], 1.0)
            for h in range(2):
                nc.vector.tensor_copy(dgd_sb[0:1, 300 * h:300 * (h + 1)],
                                      deg_psh[h][0:1, :])
            dgd = drp.tile([1, NPAD], dt.float32)
            nc.sync.dma_start(dgd[:], dgd_sb[:])
            deg_col = pool.tile([128, SCH], dt.float32)
            nc.sync.dma_start(deg_col[:],
                              dgd[:].rearrange("a (c p) -> (a p) c", p=128))
            deg_sb = pool.tile([128, SCH], dt.float32)
            nc.vector.tensor_scalar_max(deg_sb[:], deg_col[:], 1.0)
            # dinv = rsqrt(deg), newton-refined
            rc = pool.tile([128, SCH], dt.float32)
            nc.vector.reciprocal(rc[:], deg_sb[:])
            sq0 = pool.tile([128, SCH], dt.float32)
            nc.scalar.activation(sq0[:], rc[:], AF.Sqrt)
            y2 = pool.tile([128, SCH], dt.float32)
            nc.vector.tensor_mul(y2[:], sq0[:], sq0[:])
            dy2 = pool.tile([128, SCH], dt.float32)
            nc.vector.tensor_mul(dy2[:], deg_sb[:], y2[:])
            cor = pool.tile([128, SCH], dt.float32)
            nc.vector.scalar_tensor_tensor(cor[:], dy2[:], -0.5, sq0[:],
                                           ALU.mult, ALU.mult)
            dinv = pool.tile([128, SCH], dt.float32)
            nc.vector.scalar_tensor_tensor(dinv[:], sq0[:], 1.5, cor[:],
                                           ALU.mult, ALU.add)
            if debug:
                nc.sync.dma_start(dbg["d_dinv"][:], dinv[:])

            # dinv as a broadcast row [128, 640] (via DRAM bounce reshape)
            dsc = drp.tile([128, SCH], dt.float32)
            nc.sync.dma_start(dsc[:], dinv[:])
            dinvrow = pool.tile([1, NPAD], dt.float32)
            nc.sync.dma_start(dinvrow[0:1, :], dsc[:].rearrange("p c -> c p"))
            dinvb = pool.tile([128, NPAD], dt.float32)
            nc.gpsimd.partition_broadcast(dinvb[:], dinvrow[0:1, :])

            # dinv for local rows [R, 1]
            dinvloc_ps = pspA.tile([R, 1], dt.float32, tag="mm", padded_shape=[128, 512])
            for k in range(SCH):
                nc.tensor.matmul(dinvloc_ps[:], psel_sb[:, k * R:(k + 1) * R],
                                 dinv[:, k:k + 1],
                                 start=(k == 0), stop=(k == SCH - 1))
            dinvloc = pool.tile([R, 1], dt.float32)
            nc.vector.tensor_copy(dinvloc[:], dinvloc_ps[:])

            # ---------------- layer 1: F0W = X @ W  (both features), scaled
            f0w = pool.tile([128, SCH * 2 * EMB], dt.float32)  # [s, 256] chunks
            nc.vector.memset(f0w[:], 0.0)
            for m in range(SCH):
                ncols = min(128, N - 128 * m)
                fps = pspA.tile([128, 2 * EMB], dt.float32, tag="mm", padded_shape=[128, 512])
                for fi, (fsb_r, wsb_r) in enumerate(((f0sb, w0sb), (f1sb, w1sb))):
                    for k in range(F_IN // 128):
                        nc.tensor.matmul(
                            fps[0:ncols, fi * EMB:(fi + 1) * EMB],
                            fsb_r[:, k * N + 128 * m:k * N + 128 * m + ncols],
                            wsb_r[:, k * EMB:(k + 1) * EMB],
                            start=(k == 0), stop=(k == F_IN // 128 - 1))
                nc.vector.tensor_scalar_mul(
                    f0w[0:ncols, m * 2 * EMB:(m + 1) * 2 * EMB], fps[0:ncols, :],
                    dinv[0:ncols, m:m + 1])

            # ---------------- cated_T: feat-major H0/H1 + tail
            catedT = pool.tile([128, 3 * N], dt.float32)
            nc.vector.memset(catedT[:, 2 * N:3 * N], 0.0)
            nc.sync.dma_start(catedT[0:8, 2 * N:3 * N], ctail[:])
            for t in range(2):          # h0 then h1
                bcol = bf0_sb if t == 0 else bf1_sb
                for h in range(2):      # d halves of 300
                    hps = pspA.tile([128, 300], dt.float32, tag="mm", padded_shape=[128, 512])
                    for k in range(SCH):
                        nc.tensor.matmul(
                            hps[:],
                            f0w[:, k * 2 * EMB + t * EMB:
                                k * 2 * EMB + (t + 1) * EMB],
                            ain_sb[:, k * N + 300 * h:k * N + 300 * (h + 1)],
                            start=(k == 0), stop=(k == SCH - 1))
                        # lhsT = F0W chunk [s, f_t], rhs = ainT chunk [s, d]
                    hsc = wpool.tile([128, 300], dt.float32, tag="hsc")
                    nc.vector.tensor_tensor(hsc[:], hps[:],
                                            dinvb[:, 300 * h:300 * (h + 1)],
                                            ALU.mult)
                    nc.scalar.activation(
                        catedT[:, t * N + 300 * h:t * N + 300 * (h + 1)],
                        hsc[:], AF.Prelu, bias=bcol[:, 0:1], scale=1.0,
                        alpha=ALPHA_GCN)
            if debug:
                nc.sync.dma_start(dbg["d_catedT"][:], catedT[:])

            # ---------------- T1 / hl / hr in one pass over cated chunks
            t1s = pool.tile([128, SCH * FE], dt.float32)
            nc.vector.memset(t1s[:], 0.0)
            hl_nm = pool.tile([128, SCH * FE], dt.float32)
            nc.vector.memset(hl_nm[:], 0.0)
            hr_nm = wpool.tile([128, SCH * FE], dt.float32, tag="hrnm")
            nc.vector.memset(hr_nm[:], 0.0)
            for m in range(SCH):
                ncols = min(128, N - 128 * m)
                t3a = pspA.tile([128, 512], dt.float32, tag="mm", padded_shape=[128, 512])
                t3b = pspA.tile([128, FE], dt.float32, tag="mm", padded_shape=[128, 512])
                for k in range(3):
                    nc.tensor.matmul(
                        t3a[0:ncols, :],
                        catedT[:, k * N + 128 * m:k * N + 128 * m + ncols],
                        wall_sb[:, k * 768:k * 768 + 512],
                        start=(k == 0), stop=(k == 2))
                    nc.tensor.matmul(
                        t3b[0:ncols, :],
                        catedT[:, k * N + 128 * m:k * N + 128 * m + ncols],
                        wall_sb[:, k * 768 + 512:(k + 1) * 768],
                        start=(k == 0), stop=(k == 2))
                nc.vector.tensor_scalar_mul(t1s[0:ncols, m * FE:(m + 1) * FE],
                                            t3a[0:ncols, 0:FE],
                                            dinv[0:ncols, m:m + 1])
                nc.vector.tensor_copy(hl_nm[0:ncols, m * FE:(m + 1) * FE],
                                      t3a[0:ncols, FE:512])
                nc.vector.tensor_copy(hr_nm[0:ncols, m * FE:(m + 1) * FE],
                                      t3b[0:ncols, :])
            if debug:
                nc.sync.dma_start(dbg["d_t1"][:], t1s[:])

            # ---------------- hl feat-major (pairwise), cast to bf16
            hlT16 = pool.tile([128, 2 * N], dt.bfloat16)
            for t in range(2):
                for h in range(2):
                    lt_ps = pspA.tile([128, 300], dt.float32, tag="mm", padded_shape=[128, 512])
                    for k in range(3):
                        nc.tensor.matmul(
                            lt_ps[:],
                            wall_sb[:, k * 768 + FE + t * 128:
                                    k * 768 + FE + (t + 1) * 128],
                            catedT[:, k * N + 300 * h:k * N + 300 * (h + 1)],
                            start=(k == 0), stop=(k == 2))
                    nc.vector.tensor_copy(
                        hlT16[:, t * N + 300 * h:t * N + 300 * (h + 1)],
                        lt_ps[:])
            if debug:
                hlTf = wpool.tile([128, 2 * N], dt.float32, tag="hlTf")
                nc.vector.tensor_copy(hlTf[:], hlT16[:])
                nc.sync.dma_start(dbg["d_hlT"][:], hlTf[:])

            # ---------------- hr for local rows, feat-major [256, 75]
            hrloc_ps = pspA.tile([R, FE], dt.float32, tag="mm", padded_shape=[128, 512])
            for k in range(SCH):
                nc.tensor.matmul(hrloc_ps[:], psel_sb[:, k * R:(k + 1) * R],
                                 hr_nm[:, k * FE:(k + 1) * FE],
                                 start=(k == 0), stop=(k == SCH - 1))
            hrloc_nm = wpool.tile([R, FE], dt.float32, tag="hrlocnm")
            nc.vector.tensor_copy(hrloc_nm[:], hrloc_ps[:])
            hrT = pool.tile([128, 2 * R], dt.float32)   # [feat, i] 2 chunks
            for t in range(2):
                tp_ps = pspA.tile([128, R], dt.float32, tag="mm", padded_shape=[128, 512])
                nc.tensor.transpose(tp_ps[:], hrloc_nm[:, t * 128:(t + 1) * 128],
                                    ident[0:R, 0:R])
                nc.vector.tensor_copy(hrT[:, t * R:(t + 1) * R], tp_ps[:])
            if debug:
                nc.sync.dma_start(dbg["d_hr"][0:128, 0:R], hrT[:, 0:R])

            # ---------------- pairwise e: R16 = lrelu(hlT + hr_i), PE-reduce
            zwin = pool.tile([128, 2 * 63], dt.bfloat16)
            nc.vector.memset(zwin[:], 0.0)
            nc.vector.tensor_copy(zwin[:, 31:32], att_sb[:, 0:1])
            nc.vector.tensor_copy(zwin[:, 63 + 31:63 + 32], att_sb[:, 1:2])
            e_ps = [psp.tile([128, 300], dt.float32, tag=f"eps{h}",
                             name=f"eps{h}")
                    for h in range(2)]
            groups = [(0, 0, 32), (1, 32, 64), (2, 64, 75)]
            tcount = 0
            with tc.tile_pool(name="rt", bufs=6) as rtp:
                for (g, i0, i1) in groups:
                    for i in range(i0, i1):
                        rloc = i - 32 * g
                        for kt in range(2):
                            r16 = rtp.tile([128, N], dt.bfloat16, tag="r16")
                            if tcount % ACT_SHARE == ACT_SHARE - 1:
                                nc.scalar.activation(
                                    r16[:], hlT16[:, kt * N:(kt + 1) * N],
                                    AF.Prelu, bias=hrT[:, kt * R + i:kt * R + i + 1],
                                    scale=1.0, alpha=ALPHA)
                            else:
                                nc.vector._custom_dve(
                                    LRELU_BIAS, out=r16[:],
                                    in0=hlT16[:, kt * N:(kt + 1) * N],
                                    s0=hrT[:, kt * R + i:kt * R + i + 1],
                                    s1=ALPHA)
                            tcount += 1
                            for h in range(2):
                                nc.tensor.matmul(
                                    e_ps[h][32 * g:32 * g + 32, :],
                                    zwin[:, 63 * kt + 31 - rloc:
                                         63 * kt + 63 - rloc],
                                    r16[:, 300 * h:300 * (h + 1)],
                                    start=(rloc == 0 and kt == 0),
                                    stop=(rloc == (i1 - i0 - 1) and kt == 1),
                                    tile_position=(0, 32 * g))

            # ---------------- softmax over masked rows
            e_sb = pool.tile([R, N], dt.float32)
            for h in range(2):
                nc.vector.tensor_copy(e_sb[:, 300 * h:300 * (h + 1)],
                                      e_ps[h][0:R, :])
            if debug:
                nc.sync.dma_start(dbg["d_e"][:], e_sb[:])
            teq = wpool.tile([R, N], dt.float32, tag="teq")
            nc.vector.tensor_scalar(teq[:], mcr_sb[:], 0.0, None, ALU.is_equal)
            e_m = wpool.tile([R, N], dt.float32, tag="em")
            nc.vector.scalar_tensor_tensor(e_m[:], teq[:], -1e30, e_sb[:],
                                           ALU.mult, ALU.add)
            me = pool.tile([R, 1], dt.float32)
            nc.vector.tensor_reduce(me[:], e_m[:], mybir.AxisListType.X, ALU.max)
            nme = pool.tile([R, 1], dt.float32)
            nc.vector.tensor_scalar_mul(nme[:], me[:], -1.0)
            ex = wpool.tile([R, N], dt.float32, tag="ex")
            nc.scalar.activation(ex[:], e_m[:], AF.Exp, bias=nme[:, 0:1])
            exw = pool.tile([R, N], dt.float32)
            nc.vector.tensor_tensor(exw[:], ex[:], mcr_sb[:], ALU.mult)
            if debug:
                nc.sync.dma_start(dbg["d_exw"][:], exw[:])
            den = pool.tile([R, 1], dt.float32)
            nc.vector.tensor_reduce(den[:], exw[:], mybir.AxisListType.X,
                                    ALU.add)
            rec = pool.tile([R, 1], dt.float32)
            nc.vector.reciprocal(rec[:], den[:])

            # ---------------- messages: z = (exw @ hl) * rec + bgat
            exwT = pool.tile([128, SCH * R], dt.float32)
            nc.vector.memset(exwT[:], 0.0)
            for cidx in range(SCH):
                wdt = min(128, N - 128 * cidx)
                xp_ps = pspA.tile([128, R], dt.float32, tag="mm", padded_shape=[128, 512])
                nc.tensor.transpose(xp_ps[0:wdt, :],
                                    exw[:, 128 * cidx:128 * cidx + wdt],
                                    ident[0:R, 0:R])
                nc.vector.tensor_copy(exwT[0:wdt, cidx * R:(cidx + 1) * R],
                                      xp_ps[0:wdt, :])
            bgat_b = pool.tile([128, FE], dt.float32)
            nc.gpsimd.partition_broadcast(bgat_b[:], bgat_sb[0:1, :])
            z_ps = pspA.tile([R, FE], dt.float32, tag="mm", padded_shape=[128, 512])
            for cidx in range(SCH):
                nc.tensor.matmul(z_ps[:], exwT[:, cidx * R:(cidx + 1) * R],
                                 hl_nm[:, cidx * FE:(cidx + 1) * FE],
                                 start=(cidx == 0), stop=(cidx == SCH - 1))
            z_sb = pool.tile([R, FE], dt.float32)
            nc.vector.scalar_tensor_tensor(z_sb[:], z_ps[:], rec[:, 0:1],
                                           bgat_b[0:R, :], ALU.mult, ALU.add)
            if debug:
                nc.sync.dma_start(dbg["d_z"][:], z_sb[:])

            # ---------------- elu + 1
            zn = wpool.tile([R, FE], dt.float32, tag="zn")
            nc.vector.tensor_scalar_min(zn[:], z_sb[:], 0.0)
            ez = wpool.tile([R, FE], dt.float32, tag="ez")
            nc.scalar.activation(ez[:], zn[:], AF.Exp)
            g1 = pool.tile([R, FE], dt.float32)
            nc.vector.scalar_tensor_tensor(g1[:], z_sb[:], 0.0, ez[:],
                                           ALU.max, ALU.add)
            if debug:
                nc.sync.dma_start(dbg["d_g1"][:], g1[:])

            # ---------------- conv1d (K=3) + sigmoid, write into fc tile
            fc = pool.tile([R, 512], dt.float32)
            nc.vector.memset(fc[:, 510:512], 0.0)
            wb = pool.tile([128, 4], dt.float32)
            nc.gpsimd.partition_broadcast(wb[:], wcv_sb[0:1, :])
            s3 = pool.tile([128, 1], dt.float32)
            nc.vector.tensor_reduce(s3[:], wb[:, 0:3], mybir.AxisListType.X,
                                    ALU.add)
            cbp = pool.tile([128, 1], dt.float32)
            nc.vector.tensor_tensor(cbp[:], wb[:, 3:4], s3[:], ALU.subtract)
            a1 = wpool.tile([R, FE - K + 1], dt.float32, tag="a1")
            nc.vector.tensor_scalar_mul(a1[:], g1[:, 0:FE - K + 1], wb[0:R, 0:1])
            a2 = wpool.tile([R, FE - K + 1], dt.float32, tag="a2")
            nc.vector.scalar_tensor_tensor(a2[:], g1[:, 1:FE - K + 2],
                                           wb[0:R, 1:2], a1[:], ALU.mult,
                                           ALU.add)
            a3 = wpool.tile([R, FE - K + 1], dt.float32, tag="a3")
            nc.vector.scalar_tensor_tensor(a3[:], g1[:, 2:FE - K + 3],
                                           wb[0:R, 2:3], a2[:], ALU.mult,
                                           ALU.add)
            nc.scalar.activation(fc[:, FE:FE + FE - K + 1], a3[:], AF.Sigmoid,
                                 bias=cbp[0:R, 0:1])

            # ---------------- gcn_emb for local rows -> fc[:, 0:256]
            bin_b = pool.tile([128, FE], dt.float32)
            nc.gpsimd.partition_broadcast(bin_b[:], bin_sb[0:1, :])
            g_ps = pspA.tile([R, FE], dt.float32, tag="mm", padded_shape=[128, 512])
            for k in range(SCH):
                nc.tensor.matmul(g_ps[:], ainloc_sb[:, k * R:(k + 1) * R],
                                 t1s[:, k * FE:(k + 1) * FE],
                                 start=(k == 0), stop=(k == SCH - 1))
            gtmp = wpool.tile([R, FE], dt.float32, tag="gtmp")
            nc.vector.scalar_tensor_tensor(gtmp[:], g_ps[:], dinvloc[:, 0:1],
                                           bin_b[0:R, :], ALU.mult, ALU.add)
            nc.scalar.activation(fc[:, 0:FE], gtmp[:], AF.Prelu, bias=0.0,
                                 scale=1.0, alpha=ALPHA_GCN)
            if debug:
                nc.sync.dma_start(dbg["d_fc"][:], fc[:])

            # ---------------- P = fc @ W_lab
            fcT = pool.tile([128, 4 * R], dt.float32)
            for cidx in range(4):
                fp_ps = pspA.tile([128, R], dt.float32, tag="mm", padded_shape=[128, 512])
                nc.tensor.transpose(fp_ps[:],
                                    fc[:, 128 * cidx:128 * (cidx + 1)],
                                    ident[0:R, 0:R])
                nc.vector.tensor_copy(fcT[:, cidx * R:(cidx + 1) * R],
                                      fp_ps[:])
            p_ps = pspA.tile([R, C], dt.float32, tag="mm", padded_shape=[128, 512])
            for cidx in range(4):
                nc.tensor.matmul(p_ps[:], fcT[:, cidx * R:(cidx + 1) * R],
                                 wlab_sb[:, cidx * C:(cidx + 1) * C],
                                 start=(cidx == 0), stop=(cidx == 3))
            p_sb = pool.tile([R, C], dt.float32)
            nc.vector.tensor_copy(p_sb[:], p_ps[:])
            nc.sync.dma_start(pout[:], p_sb[:])

    nc.compile()
    return nc


# ================================================================ phase B ===
def build_phase_b():
    nc = bacc.Bacc("TRN2", target_bir_lowering=False, debug=False,
                   num_devices=1)
    pfull = nc.dram_tensor("pfull", [NPAD, C], dt.float32,
                           kind="ExternalInput").ap()
    afinT = nc.dram_tensor("afinT", [NPAD, N], dt.float32,
                           kind="ExternalInput").ap()
    blab = nc.dram_tensor("blab", [1, C], dt.float32,
                          kind="ExternalInput").ap()
    out = nc.dram_tensor("out", [N, C], dt.float32,
                         kind="ExternalOutput").ap()

    with tile.TileContext(nc) as tc:
        with tc.tile_pool(name="sb", bufs=1) as pool, \
             tc.tile_pool(name="ps", bufs=2, space="PSUM") as psp:
            af_sb = pool.tile([128, SCH * N], dt.float32)
            for c in range(SCH):
                nc.sync.dma_start(af_sb[:, c * N:(c + 1) * N],
                                  afinT[128 * c:128 * (c + 1), :])
            p_sb = pool.tile([128, SCH * C], dt.float32)
            for c in range(SCH):
                nc.sync.dma_start(p_sb[:, c * C:(c + 1) * C],
                                  pfull[128 * c:128 * (c + 1), :])
            blab_sb = pool.tile([1, C], dt.float32)
            nc.sync.dma_start(blab_sb[:], blab[:])
            ones_col = pool.tile([128, 1], dt.float32)
            nc.vector.memset(ones_col[:], 1.0)

            deg_ps = psp.tile([128, SCH], dt.float32, tag="mm", padded_shape=[128, 512])
            for m in range(SCH):
                nrow = min(128, N - 128 * m)
                for k in range(SCH):
                    nc.tensor.matmul(
                        deg_ps[0:nrow, m:m + 1],
                        af_sb[:, k * N + 128 * m:k * N + 128 * m + nrow],
                        ones_col[:],
                        start=(k == 0), stop=(k == SCH - 1))
            deg_sb = pool.tile([128, SCH], dt.float32)
            nc.vector.tensor_scalar_max(deg_sb[:], deg_ps[:], 1.0)
            rc = pool.tile([128, SCH], dt.float32)
            nc.vector.reciprocal(rc[:], deg_sb[:])
            sq0 = pool.tile([128, SCH], dt.float32)
            nc.scalar.activation(sq0[:], rc[:], AF.Sqrt)
            y2 = pool.tile([128, SCH], dt.float32)
            nc.vector.tensor_mul(y2[:], sq0[:], sq0[:])
            dy2 = pool.tile([128, SCH], dt.float32)
            nc.vector.tensor_mul(dy2[:], deg_sb[:], y2[:])
            cor = pool.tile([128, SCH], dt.float32)
            nc.vector.scalar_tensor_tensor(cor[:], dy2[:], -0.5, sq0[:],
                                           ALU.mult, ALU.mult)
            dinv = pool.tile([128, SCH], dt.float32)
            nc.vector.scalar_tensor_tensor(dinv[:], sq0[:], 1.5, cor[:],
                                           ALU.mult, ALU.add)

            ps_sc = pool.tile([128, SCH * C], dt.float32)
            for c in range(SCH):
                nc.vector.tensor_scalar(ps_sc[:, c * C:(c + 1) * C],
                                        p_sb[:, c * C:(c + 1) * C],
                                        dinv[:, c:c + 1], None, ALU.mult)
            blab_b = pool.tile([128, C], dt.float32)
            nc.gpsimd.partition_broadcast(blab_b[:], blab_sb[0:1, :])
            for m in range(SCH):
                nrow = min(128, N - 128 * m)
                o_ps = psp.tile([128, C], dt.float32, tag="mm", padded_shape=[128, 512])
                for k in range(SCH):
                    nc.tensor.matmul(
                        o_ps[0:nrow, :],
                        af_sb[:, k * N + 128 * m:k * N + 128 * m + nrow],
                        ps_sc[:, k * C:(k + 1) * C],
                        start=(k == 0), stop=(k == SCH - 1))
                o_sb = pool.tile([128, C], dt.float32, tag="osb", bufs=2)
                nc.vector.scalar_tensor_tensor(o_sb[0:nrow, :], o_ps[0:nrow, :],
                                               dinv[0:nrow, m:m + 1],
                                               blab_b[0:nrow, :],
                                               ALU.mult, ALU.add)
                nc.sync.dma_start(out[128 * m:128 * m + nrow, :],
                                  o_sb[0:nrow, :])
    nc.compile()
    return nc


# ============================================================ host prep ====
def _dense_count(src, dst, add_eye=True):
    """COO edge list -> dense count matrix M[src, dst] (format conversion)."""
    m = np.zeros((NPAD, N), np.float32)
    np.add.at(m, (np.asarray(src, np.int64), np.asarray(dst, np.int64)), 1.0)
    if add_eye:
        m[np.arange(N), np.arange(N)] += 1.0
    return m


def _pad_rows(a, rows):
    out = np.zeros((rows, a.shape[1]), np.float32)
    out[:a.shape[0]] = a
    return out


_CACHE = {}


def _get_programs(debug=False):
    key = ("progs", debug)
    if key not in _CACHE:
        _CACHE[key] = (build_phase_a(debug=debug), build_phase_b())
    return _CACHE[key]


def prep_a_inputs(inputs):
    f32 = np.float32
    feats0 = np.asarray(inputs["features0"], f32)
    feats1 = np.asarray(inputs["features1"], f32)
    labels = np.asarray(inputs["labels"])
    ainT = _dense_count(inputs["in_src"], inputs["in_dst"])
    mcr_full = np.zeros((N, N), f32)
    np.add.at(mcr_full, (np.asarray(inputs["cr_dst"], np.int64),
                         np.asarray(inputs["cr_src"], np.int64)), 1.0)
    mcr_full[np.arange(N), np.arange(N)] += 1.0

    onehot = np.zeros((C, N), f32)
    sup = np.asarray(labels[:NS], np.int64)
    onehot[sup, np.arange(NS)] = 1.0
    onehot[:, NS:] = 1.0 / C
    ctail = np.zeros((8, N), f32)
    ctail[0:C] = onehot
    ctail[C] = 1.0

    def wstack(w, b=None):
        out = np.zeros((384, FE), f32)
        out[0:CAT] = np.asarray(w, f32)
        if b is not None:
            out[CAT] = np.asarray(b, f32)
        return out

    att2 = np.stack([np.asarray(inputs["att_a"][0:128], f32),
                     np.asarray(inputs["att_a"][128:256], f32)], axis=1)
    wcv = np.concatenate([np.asarray(inputs["conv_w"], f32).reshape(-1),
                          np.asarray(inputs["conv_b"], f32).reshape(-1)])
    wlab = np.zeros((512, C), f32)
    wlab[0:FC] = np.asarray(inputs["W_lab"], f32)

    base = {
        "f0T": np.ascontiguousarray(feats0.T),
        "f1T": np.ascontiguousarray(feats1.T),
        "w0": np.asarray(inputs["W_f0"], f32),
        "w1": np.asarray(inputs["W_f1"], f32),
        "ainT": ainT,
        "ctail": ctail,
        "wall": np.concatenate([wstack(inputs["W_in"]),
                                wstack(inputs["Wl"], inputs["bl"]),
                                wstack(inputs["Wr"], inputs["br"])], axis=1),
        "att": att2,
        "bf0": np.asarray(inputs["b_f0"], f32).reshape(EMB, 1),
        "bf1": np.asarray(inputs["b_f1"], f32).reshape(EMB, 1),
        "bin": np.asarray(inputs["b_in"], f32).reshape(1, FE),
        "bgat": np.asarray(inputs["b_gat"], f32).reshape(1, FE),
        "wcv": wcv.reshape(1, 4),
        "wlab": wlab,
    }
    in_maps = []
    eye_sel = np.zeros((NPAD, R), f32)
    for cix in range(NCORES):
        m = dict(base)
        rows = slice(R * cix, R * (cix + 1))
        m["ainTloc"] = np.ascontiguousarray(ainT[:, rows])
        m["mcr"] = np.ascontiguousarray(mcr_full[rows, :])
        sel = np.zeros((NPAD, R), f32)
        sel[np.arange(R * cix, R * (cix + 1)), np.arange(R)] = 1.0
        m["psel"] = sel
        in_maps.append(m)
    return in_maps


def kernel(**inputs) -> np.ndarray:
    nca, ncb = _get_programs()
    in_maps = prep_a_inputs(inputs)
    res_a = run_bass_kernel_spmd(nca, in_maps, core_ids=list(range(NCORES)))
    p_full = np.concatenate([res_a.results[cix]["pout"]
                             for cix in range(NCORES)], axis=0)
    afinT = _dense_count(inputs["fin_src"], inputs["fin_dst"])
    in_b = {
        "pfull": _pad_rows(p_full, NPAD),
        "afinT": afinT,
        "blab": np.asarray(inputs["b_lab"], np.float32).reshape(1, C),
    }
    res_b = run_bass_kernel_spmd(ncb, [in_b], core_ids=[0])
    return np.asarray(res_b.results[0]["out"], np.float32)
